# revision 3
# baseline (speedup 1.0000x reference)
"""DTVNet TV-prox cascade kernel for 8 Trainium2 NeuronCores (v2).

Decomposition (hardcoded for image/sino of shape [2, 256, 256, 128] f32):
  - Data-parallel shard along D (axis 1): core k owns D slices
    [32k, 32k+32); each core gets a 38-slab chunk (halo 3 per side,
    zero-padded at global edges). Halo 3 is exact for 3 cascades: the
    domain of dependence grows by 1 slab/cascade; dual p is masked to 0
    at out-of-domain slabs (gd<0) and at global d=255.
  - On-core layout [W=128 partitions, ND, F]; 4 chunks per core:
    (b, h-half) with h-halo 3 (F=131 columns each).
  - Mixed precision: t and z are fp32 (t accumulated in fp32 PSUM and
    copied out by ScalarE; image DMA'd in fp32 - fp16 staging of the
    image alone costs 1.2e-2 rel err); duals p/q/s~, diffs and zn are
    fp16 (doubles DVE throughput on adds/clips).
  - Engine split per cascade: DVE does z/diffs/p-chain/zn clips; GPSIMD
    does the q-chain add+clip and the s~ clip; TensorE does the W-axis
    stencils AND the whole t accumulation via shifted-AP identity
    matmuls into PSUM; ScalarE drains PSUM into SBUF.
  - Duals are stored in zero-padded tiles (zp has a leading+trailing
    zero slab, zq a leading+trailing zero column) so the adjoint
    stencils are single shifted ops with no edge fixups.
"""

import sys

import numpy as np

sys.path.insert(0, "/opt/trn_rl_repo")

_B, _D, _H, _W = 2, 256, 256, 128
_NCORES = 8
_DCH = _D // _NCORES          # 32 owned D slices per core
_HALO = 3
_ND = _DCH + 2 * _HALO        # 38 slabs incl ghosts
_F = 128 + _HALO              # 131 columns per h-half chunk
_LAMB = 0.01
_CASC = 3
_MMG = 3                      # D slabs per matmul/PSUM group (3*131*4B < 2KB bank)

_RUNNER_CACHE = {}

# chunk list: (b, h0, ow0, oh);  ow0 = owned-col start within tile
_CHUNKS = [(0, 0, 0, 0), (1, 0, 0, 0), (0, 125, 3, 128), (1, 125, 3, 128)]


def _stencil_mats():
    # m1 ("Dw"): out[p] = z[p+1] - z[p] for p < 127, 0 at p = 127.
    m1 = np.zeros((128, 128), np.float32)
    for p in range(127):
        m1[p + 1, p] = 1.0
        m1[p, p] = -1.0
    # m2: adjoint contribution with s~ = -s: out[p] = s~[p] - s~[p-1].
    m2 = np.zeros((128, 128), np.float32)
    for p in range(128):
        m2[p, p] = 1.0
        if p >= 1:
            m2[p - 1, p] = -1.0
    ident = np.eye(128, dtype=np.float32)
    return m1, m2, ident


def _build_program(sigma, repeat=1):
    import contextlib

    from concourse import bacc, mybir
    from concourse.alu_op_type import AluOpType as OP
    from concourse.tile import TileContext

    f32 = mybir.dt.float32
    f16 = mybir.dt.float16
    s0, s1, s2, s3 = [float(x) for x in sigma]
    nc = bacc.Bacc()
    img = nc.declare_dram_parameter("img", [_W, _B, _ND, _H], f32, isOutput=False)
    ssd = nc.declare_dram_parameter("ssd", [_W, _B, _ND, _H], f16, isOutput=False)
    # mats16: [I, -I, M2] fp16 (fp16 moving operands); mats32: [M1] f32
    mats16 = nc.declare_dram_parameter("mats16", [3, 128, 128], f16, isOutput=False)
    mats32 = nc.declare_dram_parameter("mats32", [1, 128, 128], f32, isOutput=False)
    maskp = nc.declare_dram_parameter("maskp", [128, _ND + 1], f16, isOutput=False)
    outs = [
        nc.declare_dram_parameter(f"out{c}", [_W, _B, _DCH, _H], f32, isOutput=True)
        for c in range(_CASC)
    ]

    def groups():
        out = []
        for g0 in range(0, _ND, _MMG):
            out.append((g0, min(_MMG, _ND - g0)))
        return out

    with TileContext(nc) as tc:
        with (
            tc.tile_pool(name="const", bufs=1) as cpool,
            tc.tile_pool(name="tp", bufs=2) as tpool,
            tc.tile_pool(name="zp32", bufs=2) as zpool,
            tc.tile_pool(name="ssp", bufs=2) as sspool,
            tc.tile_pool(name="wp", bufs=2) as wpool,
            tc.tile_pool(name="pp", bufs=2) as ppool_s,
            tc.tile_pool(name="qp", bufs=2) as qpool,
            tc.tile_pool(name="sp", bufs=2) as stpool,
            tc.tile_pool(name="ps", bufs=8, space="PSUM") as ppool,
        ):
            mb16 = cpool.tile([128, 3, 128], f16)
            for mi in range(3):
                nc.sync.dma_start(out=mb16[:, mi, :], in_=mats16[mi, :, :])
            mb32 = cpool.tile([128, 1, 128], f32)
            nc.sync.dma_start(out=mb32[:, 0, :], in_=mats32[0, :, :])
            msb = cpool.tile([128, _ND + 1], f16)
            nc.sync.dma_start(out=msb[:], in_=maskp[:])
            mi_I, mi_nI, mi_M2 = 0, 1, 2

            rep_ctx = (
                tc.For_i(0, repeat, 1) if repeat > 1 else contextlib.nullcontext()
            )
            with rep_ctx:
              for pair in (_CHUNKS[0:2], _CHUNKS[2:4]):
                tiles = {}
                for ch in pair:
                    b, h0, ow0, oh = ch
                    t32 = tpool.tile([128, _ND, _F], f32, tag="t")
                    z32 = zpool.tile([128, _ND, _F], f32, tag="z")
                    ss = sspool.tile([128, _ND, _F], f16, tag="ss")
                    w = wpool.tile([128, _ND, _F], f16, tag="w")
                    zp = ppool_s.tile([128, _ND + 1, _F], f16, tag="zp")
                    zq = qpool.tile([128, _ND, _F + 1], f16, tag="zq")
                    st = stpool.tile([128, _ND, _F], f16, tag="st")
                    nc.sync.dma_start(out=t32[:], in_=img[:, b, :, h0 : h0 + _F])
                    nc.sync.dma_start(out=ss[:], in_=ssd[:, b, :, h0 : h0 + _F])
                    nc.vector.memset(zp[:, 0:1, :], 0.0)
                    nc.vector.memset(zp[:, _ND : _ND + 1, :], 0.0)
                    nc.vector.memset(zq[:, :, 0:1], 0.0)
                    nc.vector.memset(zq[:, :, _F : _F + 1], 0.0)
                    tiles[ch] = (t32, z32, ss, w, zp, zq, st)

                for c in range(_CASC):
                    for ch in pair:
                        t32, z32, ss, w, zp, zq, st = tiles[ch]
                        # ---- z = (1-lamb)*t + ss  (fp32) ----
                        nc.vector.scalar_tensor_tensor(
                            z32[:], t32[:], 1.0 - _LAMB, ss[:], OP.mult, OP.add
                        )
                        # ---- p chain (D axis): w = z[d] - z[d+1] ----
                        nc.vector.tensor_tensor(
                            w[:, 0 : _ND - 1, :],
                            z32[:, 0 : _ND - 1, :],
                            z32[:, 1:_ND, :],
                            OP.subtract,
                        )
                        if c == 0:
                            nc.vector.tensor_scalar(
                                zp[:, 1:_ND, :],
                                w[:, 0 : _ND - 1, :],
                                -s0,
                                s0,
                                OP.max,
                                OP.min,
                            )
                        else:
                            nc.vector.tensor_tensor(
                                zp[:, 1:_ND, :],
                                zp[:, 1:_ND, :],
                                w[:, 0 : _ND - 1, :],
                                OP.add,
                            )
                            nc.vector.tensor_scalar(
                                zp[:, 1:_ND, :], zp[:, 1:_ND, :], -s0, s0,
                                OP.max, OP.min,
                            )
                        # mask out-of-domain dual slabs (gd<0 / gd=255)
                        for e0, e1 in ((1, 4), (35, 36)):
                            nc.vector.tensor_tensor(
                                zp[:, e0:e1, :],
                                zp[:, e0:e1, :],
                                msb[:, e0:e1]
                                .unsqueeze(2)
                                .broadcast_to([128, e1 - e0, _F]),
                                OP.mult,
                            )
                        # ---- q chain (H axis): w = z[h] - z[h+1] ----
                        nc.vector.tensor_tensor(
                            w[:, :, 0 : _F - 1],
                            z32[:, :, 0 : _F - 1],
                            z32[:, :, 1:_F],
                            OP.subtract,
                        )
                        if c == 0:
                            nc.gpsimd.tensor_scalar(
                                zq[:, :, 1:_F],
                                w[:, :, 0 : _F - 1],
                                -s1,
                                s1,
                                OP.max,
                                OP.min,
                            )
                        else:
                            nc.gpsimd.tensor_tensor(
                                zq[:, :, 1:_F],
                                zq[:, :, 1:_F],
                                w[:, :, 0 : _F - 1],
                                OP.add,
                            )
                            nc.gpsimd.tensor_scalar(
                                zq[:, :, 1:_F], zq[:, :, 1:_F], -s1, s1,
                                OP.max, OP.min,
                            )
                        # ---- s~ chain (W axis on TensorE) ----
                        for g0, g in groups():
                            ps = ppool.tile([128, _MMG, _F], f32, tag="ps")
                            nc.tensor.matmul(
                                ps[:, 0:g, :],
                                mb32[:, 0, :],
                                z32[:, g0 : g0 + g, :],
                                start=True,
                                stop=(c == 0),
                            )
                            if c > 0:
                                nc.tensor.matmul(
                                    ps[:, 0:g, :],
                                    mb16[:, mi_I, :],
                                    st[:, g0 : g0 + g, :],
                                    start=False,
                                    stop=True,
                                )
                            nc.scalar.copy(
                                out=st[:, g0 : g0 + g, :], in_=ps[:, 0:g, :]
                            )
                        nc.gpsimd.tensor_scalar(
                            st[:], st[:], -s2, s2, OP.max, OP.min
                        )
                        # ---- zn = clip(z) (fp16, into w after q consumed) ----
                        nc.vector.tensor_scalar(
                            w[:], z32[:], -s3, s3, OP.max, OP.min
                        )
                    for ch in pair:
                        b, h0, ow0, oh = ch
                        t32, z32, ss, w, zp, zq, st = tiles[ch]
                        # ---- t = D_d^T p + D_h^T q + M2 s~ + zn  (PSUM) ----
                        for g0, g in groups():
                            pt = ppool.tile([128, _MMG, _F], f32, tag="ps")
                            nc.tensor.matmul(
                                pt[:, 0:g, :],
                                mb16[:, mi_I, :],
                                w[:, g0 : g0 + g, :],
                                start=True,
                                stop=False,
                            )
                            nc.tensor.matmul(
                                pt[:, 0:g, :],
                                mb16[:, mi_I, :],
                                zp[:, g0 : g0 + g, :],
                                start=False,
                                stop=False,
                            )
                            nc.tensor.matmul(
                                pt[:, 0:g, :],
                                mb16[:, mi_I, :],
                                zq[:, g0 : g0 + g, 0:_F],
                                start=False,
                                stop=False,
                            )
                            nc.tensor.matmul(
                                pt[:, 0:g, :],
                                mb16[:, mi_nI, :],
                                zp[:, g0 + 1 : g0 + g + 1, :],
                                start=False,
                                stop=False,
                            )
                            nc.tensor.matmul(
                                pt[:, 0:g, :],
                                mb16[:, mi_nI, :],
                                zq[:, g0 : g0 + g, 1 : _F + 1],
                                start=False,
                                stop=False,
                            )
                            nc.tensor.matmul(
                                pt[:, 0:g, :],
                                mb16[:, mi_M2, :],
                                st[:, g0 : g0 + g, :],
                                start=False,
                                stop=True,
                            )
                            nc.scalar.copy(
                                out=t32[:, g0 : g0 + g, :], in_=pt[:, 0:g, :]
                            )
                        nc.sync.dma_start(
                            out=outs[c][:, b, :, oh : oh + 128],
                            in_=t32[:, _HALO : _HALO + _DCH, ow0 : ow0 + 128],
                        )
    nc.compile()
    return nc


def _make_runner(nc, n_cores):
    """Build a reusable (cached-jit) runner for the Bass program, modeled
    on concourse.bass2jax.run_bass_via_pjrt."""
    import jax
    from jax.experimental.shard_map import shard_map
    from jax.sharding import Mesh, PartitionSpec

    from concourse import bass2jax, mybir

    bass2jax.install_neuronx_cc_hook()

    partition_name = (
        nc.partition_id_tensor.name if nc.partition_id_tensor else None
    )
    in_names, out_names, out_avals = [], [], []
    for alloc in nc.m.functions[0].allocations:
        if not isinstance(alloc, mybir.MemoryLocationSet):
            continue
        name = alloc.memorylocations[0].name
        if alloc.kind == "ExternalInput":
            if name != partition_name:
                in_names.append(name)
        elif alloc.kind == "ExternalOutput":
            shape = tuple(alloc.tensor_shape)
            dtype = mybir.dt.np(alloc.dtype)
            out_names.append(name)
            out_avals.append(jax.core.ShapedArray(shape, dtype))
    n_params = len(in_names)
    n_outs = len(out_avals)
    all_in_names = tuple(in_names + out_names + ([partition_name] if partition_name else []))
    donate = tuple(range(n_params, n_params + n_outs))

    def _body(*args):
        operands = list(args)
        if partition_name is not None:
            operands.append(bass2jax.partition_id_tensor())
        return tuple(
            bass2jax._bass_exec_p.bind(
                *operands,
                out_avals=tuple(out_avals),
                in_names=all_in_names,
                out_names=tuple(out_names),
                lowering_input_output_aliases=(),
                sim_require_finite=True,
                sim_require_nnan=True,
                nc=nc,
            )
        )

    devices = jax.devices()[:n_cores]
    assert len(devices) == n_cores
    mesh = Mesh(np.asarray(devices), ("core",))
    in_specs = (PartitionSpec("core"),) * (n_params + n_outs)
    out_specs = (PartitionSpec("core"),) * n_outs
    sharded = jax.jit(
        shard_map(
            _body, mesh=mesh, in_specs=in_specs, out_specs=out_specs, check_rep=False
        ),
        donate_argnums=donate,
        keep_unused=True,
    )

    def _concat_inputs(in_maps):
        per_core = [[np.asarray(m[name]) for name in in_names] for m in in_maps]
        return [
            np.concatenate([per_core[c][i] for c in range(n_cores)], axis=0)
            for i in range(n_params)
        ]

    def run(in_maps):
        concat_in = _concat_inputs(in_maps)
        concat_zeros = [
            np.zeros((n_cores * a.shape[0], *a.shape[1:]), a.dtype) for a in out_avals
        ]
        out_arrs = sharded(*concat_in, *concat_zeros)
        return [
            {
                name: np.asarray(out_arrs[i]).reshape(
                    n_cores, *out_avals[i].shape
                )[c]
                for i, name in enumerate(out_names)
            }
            for c in range(n_cores)
        ]

    def time_device(in_maps, reps=20):
        """Device-exec wall time with inputs pre-staged on device and
        outputs left on device (no tunnel transfer in the timed region)."""
        import time as _time

        sharded_nodonate = jax.jit(
            shard_map(
                _body,
                mesh=mesh,
                in_specs=in_specs,
                out_specs=out_specs,
                check_rep=False,
            ),
            keep_unused=True,
        )
        from jax.sharding import NamedSharding

        concat_in = _concat_inputs(in_maps)
        concat_zeros = [
            np.zeros((n_cores * a.shape[0], *a.shape[1:]), a.dtype) for a in out_avals
        ]
        shard = NamedSharding(mesh, PartitionSpec("core"))
        dev_in = [jax.device_put(x, shard) for x in concat_in]
        dev_zero = [jax.device_put(x, shard) for x in concat_zeros]
        out = sharded_nodonate(*dev_in, *dev_zero)  # warm + compile
        jax.block_until_ready(out)
        times = []
        for _ in range(reps):
            t0 = _time.perf_counter()
            out = sharded_nodonate(*dev_in, *dev_zero)
            jax.block_until_ready(out)
            times.append(_time.perf_counter() - t0)
        return times

    run.time_device = time_device
    return run


def _get_runner(sigma):
    key = tuple(float(x) for x in np.asarray(sigma).ravel())
    if key not in _RUNNER_CACHE:
        nc = _build_program(sigma)
        _RUNNER_CACHE[key] = _make_runner(nc, _NCORES)
    return _RUNNER_CACHE[key]


def _build_in_maps(image, sino):
    from concurrent.futures import ThreadPoolExecutor

    m1, m2, ident = _stencil_mats()
    mats16 = np.stack([ident, -ident, m2]).astype(np.float16)
    mats32 = m1[None].astype(np.float32)

    def one_core(k):
        d0 = k * _DCH - _HALO
        img_c = np.zeros((_W, _B, _ND, _H), np.float32)
        ss_c = np.zeros((_W, _B, _ND, _H), np.float16)
        lo, hi = max(0, d0), min(_D, d0 + _ND)
        img_c[:, :, lo - d0 : hi - d0, :] = image[:, lo:hi].transpose(3, 0, 1, 2)
        ss_c[:, :, lo - d0 : hi - d0, :] = (
            _LAMB * sino[:, lo:hi].transpose(3, 0, 1, 2)
        ).astype(np.float16)
        # maskp[j]: dual p at gd = 32k - 4 + j; valid iff 0 <= gd <= 254
        gd = _DCH * k - 4 + np.arange(_ND + 1)
        maskp = np.broadcast_to(
            ((gd >= 0) & (gd <= _D - 2)).astype(np.float16), (128, _ND + 1)
        ).copy()
        return {
            "img": img_c,
            "ssd": ss_c,
            "mats16": mats16,
            "mats32": mats32,
            "maskp": maskp,
        }

    with ThreadPoolExecutor(max_workers=_NCORES) as ex:
        return list(ex.map(one_core, range(_NCORES)))


def _reference_numpy(image, sino, sigma, nt):
    """Slow exact fallback for unexpected inputs (e.g. nt != 0)."""
    def fwd_diff(v, ax):
        d = np.diff(v, axis=ax)
        pad = [(0, 0)] * v.ndim
        pad[ax] = (0, 1)
        return np.pad(d, pad)

    def fwd_diff_t(pp, ax):
        n = pp.shape[ax]
        pad_front = [(0, 0)] * pp.ndim
        pad_front[ax] = (1, 0)
        a = np.pad(pp, pad_front)
        a = np.take(a, range(n), axis=ax)
        pad_back = [(0, 0)] * pp.ndim
        pad_back[ax] = (0, 1)
        b = np.pad(np.take(pp, range(n - 1), axis=ax), pad_back)
        return a - b

    t = image.astype(np.float32)
    out = [t]
    p = np.zeros_like(t)
    q = np.zeros_like(t)
    s = np.zeros_like(t)
    for c in range(_CASC):
        z = t - np.float32(_LAMB) * (t - sino)
        pn = np.clip(p - fwd_diff(z, 1), -sigma[0], sigma[0])
        qn = np.clip(q - fwd_diff(z, 2), -sigma[1], sigma[1])
        sn = np.clip(s - fwd_diff(z, 3), -sigma[2], sigma[2])
        zn = np.clip(z, -sigma[3], sigma[3])
        p = pn + nt[c] * (pn - p)
        q = qn + nt[c] * (qn - q)
        s = sn + nt[c] * (sn - s)
        t = fwd_diff_t(p, 1) + fwd_diff_t(q, 2) + fwd_diff_t(s, 3) + zn
        out.append(t.astype(np.float32))
    return tuple(out)


def kernel(image, sino, sigma, nt):
    image = np.asarray(image, np.float32)
    sino = np.asarray(sino, np.float32)
    sigma = np.asarray(sigma, np.float32)
    nt = np.asarray(nt, np.float32)

    if (
        image.shape != (_B, _D, _H, _W)
        or sino.shape != (_B, _D, _H, _W)
        or np.any(nt != 0.0)
    ):
        return _reference_numpy(image, sino, sigma, nt)

    try:
        return _device_path(image, sino, sigma)
    except Exception:
        try:
            return _device_path(image, sino, sigma)  # retry: transient wedge
        except Exception:
            return _reference_numpy(image, sino, sigma, nt)


def _device_path(image, sino, sigma):
    runner = _get_runner(sigma)
    results = runner(_build_in_maps(image, sino))

    from concurrent.futures import ThreadPoolExecutor

    def gather(c):
        # per-core [W, B, DCH, H] -> concat d -> [B, D, H, W]
        cat = np.concatenate(
            [results[k][f"out{c}"] for k in range(_NCORES)], axis=2
        )
        return np.ascontiguousarray(cat.transpose(1, 2, 3, 0))

    with ThreadPoolExecutor(max_workers=_CASC) as ex:
        full = list(ex.map(gather, range(_CASC)))
    return (image, full[0], full[1], full[2])


# revision 4
# speedup vs baseline: 1.8286x; 1.8286x over previous
"""DTVNet TV-prox cascade kernel for 8 Trainium2 NeuronCores (v2).

Decomposition (hardcoded for image/sino of shape [2, 256, 256, 128] f32):
  - Data-parallel shard along D (axis 1): core k owns D slices
    [32k, 32k+32); each core gets a 38-slab chunk (halo 3 per side,
    zero-padded at global edges). Halo 3 is exact for 3 cascades: the
    domain of dependence grows by 1 slab/cascade; dual p is masked to 0
    at out-of-domain slabs (gd<0) and at global d=255.
  - On-core layout [W=128 partitions, ND, F]; 4 chunks per core:
    (b, h-half) with h-halo 3 (F=131 columns each).
  - Mixed precision: t and z are fp32 (t accumulated in fp32 PSUM and
    copied out by ScalarE; image DMA'd in fp32 - fp16 staging of the
    image alone costs 1.2e-2 rel err); duals p/q/s~, diffs and zn are
    fp16 (doubles DVE throughput on adds/clips).
  - Engine split per cascade: DVE does z/diffs/p-chain/zn clips; GPSIMD
    does the q-chain add+clip and the s~ clip; TensorE does the W-axis
    stencils AND the whole t accumulation via shifted-AP identity
    matmuls into PSUM; ScalarE drains PSUM into SBUF.
  - Duals are stored in zero-padded tiles (zp has a leading+trailing
    zero slab, zq a leading+trailing zero column) so the adjoint
    stencils are single shifted ops with no edge fixups.
"""

import sys

import numpy as np

sys.path.insert(0, "/opt/trn_rl_repo")

_B, _D, _H, _W = 2, 256, 256, 128
_NCORES = 8
_DCH = _D // _NCORES          # 32 owned D slices per core
_HALO = 3
_ND = _DCH + 2 * _HALO        # 38 slabs incl ghosts
_F = 128 + _HALO              # 131 columns per h-half chunk
_LAMB = 0.01
_CASC = 3
_MMG = 3                      # D slabs per matmul/PSUM group (3*131*4B < 2KB bank)

_RUNNER_CACHE = {}

# chunk list: (b, h0, ow0, oh);  ow0 = owned-col start within tile
_CHUNKS = [(0, 0, 0, 0), (1, 0, 0, 0), (0, 125, 3, 128), (1, 125, 3, 128)]


def _stencil_mats():
    # m1 ("Dw"): out[p] = z[p+1] - z[p] for p < 127, 0 at p = 127.
    m1 = np.zeros((128, 128), np.float32)
    for p in range(127):
        m1[p + 1, p] = 1.0
        m1[p, p] = -1.0
    # m2: adjoint contribution with s~ = -s: out[p] = s~[p] - s~[p-1].
    m2 = np.zeros((128, 128), np.float32)
    for p in range(128):
        m2[p, p] = 1.0
        if p >= 1:
            m2[p - 1, p] = -1.0
    ident = np.eye(128, dtype=np.float32)
    return m1, m2, ident


def _build_program(sigma, repeat=1):
    import contextlib

    from concourse import bacc, mybir
    from concourse.alu_op_type import AluOpType as OP
    from concourse.tile import TileContext

    f32 = mybir.dt.float32
    f16 = mybir.dt.float16
    s0, s1, s2, s3 = [float(x) for x in sigma]
    nc = bacc.Bacc()
    img = nc.declare_dram_parameter("img", [_W, _B, _ND, _H], f32, isOutput=False)
    ssd = nc.declare_dram_parameter("ssd", [_W, _B, _ND, _H], f16, isOutput=False)
    # mats16: [I, -I, M2] fp16 (fp16 moving operands); mats32: [M1] f32
    mats16 = nc.declare_dram_parameter("mats16", [3, 128, 128], f16, isOutput=False)
    mats32 = nc.declare_dram_parameter("mats32", [1, 128, 128], f32, isOutput=False)
    maskp = nc.declare_dram_parameter("maskp", [128, _ND + 1], f16, isOutput=False)
    outs = [
        nc.declare_dram_parameter(f"out{c}", [_W, _B, _DCH, _H], f32, isOutput=True)
        for c in range(_CASC)
    ]

    def groups():
        out = []
        for g0 in range(0, _ND, _MMG):
            out.append((g0, min(_MMG, _ND - g0)))
        return out

    with TileContext(nc) as tc:
        with (
            tc.tile_pool(name="const", bufs=1) as cpool,
            tc.tile_pool(name="tp", bufs=2) as tpool,
            tc.tile_pool(name="zp32", bufs=2) as zpool,
            tc.tile_pool(name="ssp", bufs=2) as sspool,
            tc.tile_pool(name="wp", bufs=2) as wpool,
            tc.tile_pool(name="pp", bufs=2) as ppool_s,
            tc.tile_pool(name="qp", bufs=2) as qpool,
            tc.tile_pool(name="sp", bufs=2) as stpool,
            tc.tile_pool(name="ps", bufs=8, space="PSUM") as ppool,
        ):
            mb16 = cpool.tile([128, 3, 128], f16)
            for mi in range(3):
                nc.sync.dma_start(out=mb16[:, mi, :], in_=mats16[mi, :, :])
            mb32 = cpool.tile([128, 1, 128], f32)
            nc.sync.dma_start(out=mb32[:, 0, :], in_=mats32[0, :, :])
            msb = cpool.tile([128, _ND + 1], f16)
            nc.sync.dma_start(out=msb[:], in_=maskp[:])
            mi_I, mi_nI, mi_M2 = 0, 1, 2

            rep_ctx = (
                tc.For_i(0, repeat, 1) if repeat > 1 else contextlib.nullcontext()
            )
            with rep_ctx:
              for pair in (_CHUNKS[0:2], _CHUNKS[2:4]):
                tiles = {}
                for ch in pair:
                    b, h0, ow0, oh = ch
                    t32 = tpool.tile([128, _ND, _F], f32, tag="t")
                    z32 = zpool.tile([128, _ND, _F], f32, tag="z")
                    ss = sspool.tile([128, _ND, _F], f16, tag="ss")
                    w = wpool.tile([128, _ND, _F], f16, tag="w")
                    zp = ppool_s.tile([128, _ND + 1, _F], f16, tag="zp")
                    zq = qpool.tile([128, _ND, _F + 1], f16, tag="zq")
                    st = stpool.tile([128, _ND, _F], f16, tag="st")
                    nc.sync.dma_start(out=t32[:], in_=img[:, b, :, h0 : h0 + _F])
                    nc.sync.dma_start(out=ss[:], in_=ssd[:, b, :, h0 : h0 + _F])
                    nc.vector.memset(zp[:, 0:1, :], 0.0)
                    nc.vector.memset(zp[:, _ND : _ND + 1, :], 0.0)
                    nc.vector.memset(zq[:, :, 0:1], 0.0)
                    nc.vector.memset(zq[:, :, _F : _F + 1], 0.0)
                    tiles[ch] = (t32, z32, ss, w, zp, zq, st)

                for c in range(_CASC):
                    for ch in pair:
                        t32, z32, ss, w, zp, zq, st = tiles[ch]
                        # ---- z = (1-lamb)*t + ss  (fp32) ----
                        nc.vector.scalar_tensor_tensor(
                            z32[:], t32[:], 1.0 - _LAMB, ss[:], OP.mult, OP.add
                        )
                        # ---- p chain (D axis): w = z[d] - z[d+1] ----
                        nc.vector.tensor_tensor(
                            w[:, 0 : _ND - 1, :],
                            z32[:, 0 : _ND - 1, :],
                            z32[:, 1:_ND, :],
                            OP.subtract,
                        )
                        if c == 0:
                            nc.vector.tensor_scalar(
                                zp[:, 1:_ND, :],
                                w[:, 0 : _ND - 1, :],
                                -s0,
                                s0,
                                OP.max,
                                OP.min,
                            )
                        else:
                            nc.vector.tensor_tensor(
                                zp[:, 1:_ND, :],
                                zp[:, 1:_ND, :],
                                w[:, 0 : _ND - 1, :],
                                OP.add,
                            )
                            nc.vector.tensor_scalar(
                                zp[:, 1:_ND, :], zp[:, 1:_ND, :], -s0, s0,
                                OP.max, OP.min,
                            )
                        # mask out-of-domain dual slabs (gd<0 / gd=255)
                        for e0, e1 in ((1, 4), (35, 36)):
                            nc.vector.tensor_tensor(
                                zp[:, e0:e1, :],
                                zp[:, e0:e1, :],
                                msb[:, e0:e1]
                                .unsqueeze(2)
                                .broadcast_to([128, e1 - e0, _F]),
                                OP.mult,
                            )
                        # ---- q chain (H axis): w = z[h] - z[h+1] ----
                        nc.vector.tensor_tensor(
                            w[:, :, 0 : _F - 1],
                            z32[:, :, 0 : _F - 1],
                            z32[:, :, 1:_F],
                            OP.subtract,
                        )
                        if c == 0:
                            nc.vector.tensor_scalar(
                                zq[:, :, 1:_F],
                                w[:, :, 0 : _F - 1],
                                -s1,
                                s1,
                                OP.max,
                                OP.min,
                            )
                        else:
                            nc.vector.tensor_tensor(
                                zq[:, :, 1:_F],
                                zq[:, :, 1:_F],
                                w[:, :, 0 : _F - 1],
                                OP.add,
                            )
                            nc.vector.tensor_scalar(
                                zq[:, :, 1:_F], zq[:, :, 1:_F], -s1, s1,
                                OP.max, OP.min,
                            )
                        # ---- s~ chain (W axis on TensorE) ----
                        for g0, g in groups():
                            ps = ppool.tile([128, _MMG, _F], f32, tag="ps")
                            nc.tensor.matmul(
                                ps[:, 0:g, :],
                                mb32[:, 0, :],
                                z32[:, g0 : g0 + g, :],
                                start=True,
                                stop=(c == 0),
                            )
                            if c > 0:
                                nc.tensor.matmul(
                                    ps[:, 0:g, :],
                                    mb16[:, mi_I, :],
                                    st[:, g0 : g0 + g, :],
                                    start=False,
                                    stop=True,
                                )
                            nc.scalar.copy(
                                out=st[:, g0 : g0 + g, :], in_=ps[:, 0:g, :]
                            )
                        nc.vector.tensor_scalar(
                            st[:], st[:], -s2, s2, OP.max, OP.min
                        )
                        # ---- zn = clip(z) (fp16, into w after q consumed) ----
                        nc.vector.tensor_scalar(
                            w[:], z32[:], -s3, s3, OP.max, OP.min
                        )
                    for ch in pair:
                        b, h0, ow0, oh = ch
                        t32, z32, ss, w, zp, zq, st = tiles[ch]
                        # ---- t = D_d^T p + D_h^T q + M2 s~ + zn  (PSUM) ----
                        for g0, g in groups():
                            pt = ppool.tile([128, _MMG, _F], f32, tag="ps")
                            nc.tensor.matmul(
                                pt[:, 0:g, :],
                                mb16[:, mi_I, :],
                                w[:, g0 : g0 + g, :],
                                start=True,
                                stop=False,
                            )
                            nc.tensor.matmul(
                                pt[:, 0:g, :],
                                mb16[:, mi_I, :],
                                zp[:, g0 : g0 + g, :],
                                start=False,
                                stop=False,
                            )
                            nc.tensor.matmul(
                                pt[:, 0:g, :],
                                mb16[:, mi_I, :],
                                zq[:, g0 : g0 + g, 0:_F],
                                start=False,
                                stop=False,
                            )
                            nc.tensor.matmul(
                                pt[:, 0:g, :],
                                mb16[:, mi_nI, :],
                                zp[:, g0 + 1 : g0 + g + 1, :],
                                start=False,
                                stop=False,
                            )
                            nc.tensor.matmul(
                                pt[:, 0:g, :],
                                mb16[:, mi_nI, :],
                                zq[:, g0 : g0 + g, 1 : _F + 1],
                                start=False,
                                stop=False,
                            )
                            nc.tensor.matmul(
                                pt[:, 0:g, :],
                                mb16[:, mi_M2, :],
                                st[:, g0 : g0 + g, :],
                                start=False,
                                stop=True,
                            )
                            nc.scalar.copy(
                                out=t32[:, g0 : g0 + g, :], in_=pt[:, 0:g, :]
                            )
                        nc.sync.dma_start(
                            out=outs[c][:, b, :, oh : oh + 128],
                            in_=t32[:, _HALO : _HALO + _DCH, ow0 : ow0 + 128],
                        )
    nc.compile()
    return nc


def _make_runner(nc, n_cores):
    """Build a reusable (cached-jit) runner for the Bass program, modeled
    on concourse.bass2jax.run_bass_via_pjrt."""
    import jax
    from jax.experimental.shard_map import shard_map
    from jax.sharding import Mesh, PartitionSpec

    from concourse import bass2jax, mybir

    bass2jax.install_neuronx_cc_hook()

    partition_name = (
        nc.partition_id_tensor.name if nc.partition_id_tensor else None
    )
    in_names, out_names, out_avals = [], [], []
    for alloc in nc.m.functions[0].allocations:
        if not isinstance(alloc, mybir.MemoryLocationSet):
            continue
        name = alloc.memorylocations[0].name
        if alloc.kind == "ExternalInput":
            if name != partition_name:
                in_names.append(name)
        elif alloc.kind == "ExternalOutput":
            shape = tuple(alloc.tensor_shape)
            dtype = mybir.dt.np(alloc.dtype)
            out_names.append(name)
            out_avals.append(jax.core.ShapedArray(shape, dtype))
    n_params = len(in_names)
    n_outs = len(out_avals)
    all_in_names = tuple(in_names + out_names + ([partition_name] if partition_name else []))
    donate = tuple(range(n_params, n_params + n_outs))

    def _body(*args):
        operands = list(args)
        if partition_name is not None:
            operands.append(bass2jax.partition_id_tensor())
        return tuple(
            bass2jax._bass_exec_p.bind(
                *operands,
                out_avals=tuple(out_avals),
                in_names=all_in_names,
                out_names=tuple(out_names),
                lowering_input_output_aliases=(),
                sim_require_finite=True,
                sim_require_nnan=True,
                nc=nc,
            )
        )

    devices = jax.devices()[:n_cores]
    assert len(devices) == n_cores
    mesh = Mesh(np.asarray(devices), ("core",))
    in_specs = (PartitionSpec("core"),) * (n_params + n_outs)
    out_specs = (PartitionSpec("core"),) * n_outs
    sharded = jax.jit(
        shard_map(
            _body, mesh=mesh, in_specs=in_specs, out_specs=out_specs, check_rep=False
        ),
        donate_argnums=donate,
        keep_unused=True,
    )

    def _concat_inputs(in_maps):
        per_core = [[np.asarray(m[name]) for name in in_names] for m in in_maps]
        return [
            np.concatenate([per_core[c][i] for c in range(n_cores)], axis=0)
            for i in range(n_params)
        ]

    def run(in_maps):
        concat_in = _concat_inputs(in_maps)
        concat_zeros = [
            np.zeros((n_cores * a.shape[0], *a.shape[1:]), a.dtype) for a in out_avals
        ]
        out_arrs = sharded(*concat_in, *concat_zeros)
        return [
            {
                name: np.asarray(out_arrs[i]).reshape(
                    n_cores, *out_avals[i].shape
                )[c]
                for i, name in enumerate(out_names)
            }
            for c in range(n_cores)
        ]

    def time_device(in_maps, reps=20):
        """Device-exec wall time with inputs pre-staged on device and
        outputs left on device (no tunnel transfer in the timed region)."""
        import time as _time

        sharded_nodonate = jax.jit(
            shard_map(
                _body,
                mesh=mesh,
                in_specs=in_specs,
                out_specs=out_specs,
                check_rep=False,
            ),
            keep_unused=True,
        )
        from jax.sharding import NamedSharding

        concat_in = _concat_inputs(in_maps)
        concat_zeros = [
            np.zeros((n_cores * a.shape[0], *a.shape[1:]), a.dtype) for a in out_avals
        ]
        shard = NamedSharding(mesh, PartitionSpec("core"))
        dev_in = [jax.device_put(x, shard) for x in concat_in]
        dev_zero = [jax.device_put(x, shard) for x in concat_zeros]
        out = sharded_nodonate(*dev_in, *dev_zero)  # warm + compile
        jax.block_until_ready(out)
        times = []
        for _ in range(reps):
            t0 = _time.perf_counter()
            out = sharded_nodonate(*dev_in, *dev_zero)
            jax.block_until_ready(out)
            times.append(_time.perf_counter() - t0)
        return times

    run.time_device = time_device
    return run


def _get_runner(sigma):
    key = tuple(float(x) for x in np.asarray(sigma).ravel())
    if key not in _RUNNER_CACHE:
        nc = _build_program(sigma)
        _RUNNER_CACHE[key] = _make_runner(nc, _NCORES)
    return _RUNNER_CACHE[key]


def _build_in_maps(image, sino):
    from concurrent.futures import ThreadPoolExecutor

    m1, m2, ident = _stencil_mats()
    mats16 = np.stack([ident, -ident, m2]).astype(np.float16)
    mats32 = m1[None].astype(np.float32)

    def one_core(k):
        d0 = k * _DCH - _HALO
        img_c = np.zeros((_W, _B, _ND, _H), np.float32)
        ss_c = np.zeros((_W, _B, _ND, _H), np.float16)
        lo, hi = max(0, d0), min(_D, d0 + _ND)
        img_c[:, :, lo - d0 : hi - d0, :] = image[:, lo:hi].transpose(3, 0, 1, 2)
        ss_c[:, :, lo - d0 : hi - d0, :] = (
            _LAMB * sino[:, lo:hi].transpose(3, 0, 1, 2)
        ).astype(np.float16)
        # maskp[j]: dual p at gd = 32k - 4 + j; valid iff 0 <= gd <= 254
        gd = _DCH * k - 4 + np.arange(_ND + 1)
        maskp = np.broadcast_to(
            ((gd >= 0) & (gd <= _D - 2)).astype(np.float16), (128, _ND + 1)
        ).copy()
        return {
            "img": img_c,
            "ssd": ss_c,
            "mats16": mats16,
            "mats32": mats32,
            "maskp": maskp,
        }

    with ThreadPoolExecutor(max_workers=_NCORES) as ex:
        return list(ex.map(one_core, range(_NCORES)))


def _reference_numpy(image, sino, sigma, nt):
    """Slow exact fallback for unexpected inputs (e.g. nt != 0)."""
    def fwd_diff(v, ax):
        d = np.diff(v, axis=ax)
        pad = [(0, 0)] * v.ndim
        pad[ax] = (0, 1)
        return np.pad(d, pad)

    def fwd_diff_t(pp, ax):
        n = pp.shape[ax]
        pad_front = [(0, 0)] * pp.ndim
        pad_front[ax] = (1, 0)
        a = np.pad(pp, pad_front)
        a = np.take(a, range(n), axis=ax)
        pad_back = [(0, 0)] * pp.ndim
        pad_back[ax] = (0, 1)
        b = np.pad(np.take(pp, range(n - 1), axis=ax), pad_back)
        return a - b

    t = image.astype(np.float32)
    out = [t]
    p = np.zeros_like(t)
    q = np.zeros_like(t)
    s = np.zeros_like(t)
    for c in range(_CASC):
        z = t - np.float32(_LAMB) * (t - sino)
        pn = np.clip(p - fwd_diff(z, 1), -sigma[0], sigma[0])
        qn = np.clip(q - fwd_diff(z, 2), -sigma[1], sigma[1])
        sn = np.clip(s - fwd_diff(z, 3), -sigma[2], sigma[2])
        zn = np.clip(z, -sigma[3], sigma[3])
        p = pn + nt[c] * (pn - p)
        q = qn + nt[c] * (qn - q)
        s = sn + nt[c] * (sn - s)
        t = fwd_diff_t(p, 1) + fwd_diff_t(q, 2) + fwd_diff_t(s, 3) + zn
        out.append(t.astype(np.float32))
    return tuple(out)


def kernel(image, sino, sigma, nt):
    image = np.asarray(image, np.float32)
    sino = np.asarray(sino, np.float32)
    sigma = np.asarray(sigma, np.float32)
    nt = np.asarray(nt, np.float32)

    if (
        image.shape != (_B, _D, _H, _W)
        or sino.shape != (_B, _D, _H, _W)
        or np.any(nt != 0.0)
    ):
        return _reference_numpy(image, sino, sigma, nt)

    try:
        return _device_path(image, sino, sigma)
    except Exception:
        try:
            return _device_path(image, sino, sigma)  # retry: transient wedge
        except Exception:
            return _reference_numpy(image, sino, sigma, nt)


def _device_path(image, sino, sigma):
    runner = _get_runner(sigma)
    results = runner(_build_in_maps(image, sino))

    from concurrent.futures import ThreadPoolExecutor

    def gather(c):
        # per-core [W, B, DCH, H] -> concat d -> [B, D, H, W]
        cat = np.concatenate(
            [results[k][f"out{c}"] for k in range(_NCORES)], axis=2
        )
        return np.ascontiguousarray(cat.transpose(1, 2, 3, 0))

    with ThreadPoolExecutor(max_workers=_CASC) as ex:
        full = list(ex.map(gather, range(_CASC)))
    return (image, full[0], full[1], full[2])


# revision 5
# speedup vs baseline: 2.7677x; 1.5135x over previous
"""DTVNet TV-prox cascade kernel for 8 Trainium2 NeuronCores (v2).

Decomposition (hardcoded for image/sino of shape [2, 256, 256, 128] f32):
  - Data-parallel shard along D (axis 1): core k owns D slices
    [32k, 32k+32); each core gets a 38-slab chunk (halo 3 per side,
    zero-padded at global edges). Halo 3 is exact for 3 cascades: the
    domain of dependence grows by 1 slab/cascade; dual p is masked to 0
    at out-of-domain slabs (gd<0) and at global d=255.
  - On-core layout [W=128 partitions, ND, F]; 4 chunks per core:
    (b, h-half) with h-halo 3 (F=131 columns each).
  - Mixed precision: t and z are fp32 (t accumulated in fp32 PSUM and
    copied out by ScalarE; image DMA'd in fp32 - fp16 staging of the
    image alone costs 1.2e-2 rel err); duals p/q/s~, diffs and zn are
    fp16 (doubles DVE throughput on adds/clips).
  - Engine split per cascade: DVE does z/diffs/p-chain/zn clips; GPSIMD
    does the q-chain add+clip and the s~ clip; TensorE does the W-axis
    stencils AND the whole t accumulation via shifted-AP identity
    matmuls into PSUM; ScalarE drains PSUM into SBUF.
  - Duals are stored in zero-padded tiles (zp has a leading+trailing
    zero slab, zq a leading+trailing zero column) so the adjoint
    stencils are single shifted ops with no edge fixups.
"""

import sys

import numpy as np

sys.path.insert(0, "/opt/trn_rl_repo")

_B, _D, _H, _W = 2, 256, 256, 128
_NCORES = 8
_DCH = _D // _NCORES          # 32 owned D slices per core
_HALO = 3
_ND = _DCH + 2 * _HALO        # 38 slabs incl ghosts
_F = 128 + _HALO              # 131 columns per h-half chunk
_LAMB = 0.01
_CASC = 3
_MMG = 3                      # D slabs per matmul/PSUM group (3*131*4B < 2KB bank)

_RUNNER_CACHE = {}

# chunk list: (b, h0, ow0, oh);  ow0 = owned-col start within tile
_CHUNKS = [(0, 0, 0, 0), (1, 0, 0, 0), (0, 125, 3, 128), (1, 125, 3, 128)]


def _stencil_mats():
    # m1 ("Dw"): out[p] = z[p+1] - z[p] for p < 127, 0 at p = 127.
    m1 = np.zeros((128, 128), np.float32)
    for p in range(127):
        m1[p + 1, p] = 1.0
        m1[p, p] = -1.0
    # m2: adjoint contribution with s~ = -s: out[p] = s~[p] - s~[p-1].
    m2 = np.zeros((128, 128), np.float32)
    for p in range(128):
        m2[p, p] = 1.0
        if p >= 1:
            m2[p - 1, p] = -1.0
    ident = np.eye(128, dtype=np.float32)
    return m1, m2, ident


def _build_program(sigma, repeat=1):
    import contextlib

    from concourse import bacc, mybir
    from concourse.alu_op_type import AluOpType as OP
    from concourse.tile import TileContext

    f32 = mybir.dt.float32
    f16 = mybir.dt.float16
    s0, s1, s2, s3 = [float(x) for x in sigma]
    nc = bacc.Bacc()
    img = nc.declare_dram_parameter("img", [_W, _B, _ND, _H], f32, isOutput=False)
    ssd = nc.declare_dram_parameter("ssd", [_W, _B, _ND, _H], f16, isOutput=False)
    # mats16: [I, -I, M2] fp16 (fp16 moving operands); mats32: [M1] f32
    mats16 = nc.declare_dram_parameter("mats16", [3, 128, 128], f16, isOutput=False)
    mats32 = nc.declare_dram_parameter("mats32", [1, 128, 128], f32, isOutput=False)
    maskp = nc.declare_dram_parameter("maskp", [128, _ND + 1], f16, isOutput=False)
    outs = [
        nc.declare_dram_parameter(f"out{c}", [_W, _B, _DCH, _H], f32, isOutput=True)
        for c in range(_CASC)
    ]

    def groups():
        out = []
        for g0 in range(0, _ND, _MMG):
            out.append((g0, min(_MMG, _ND - g0)))
        return out

    with TileContext(nc) as tc:
        with (
            tc.tile_pool(name="const", bufs=1) as cpool,
            tc.tile_pool(name="tp", bufs=2) as tpool,
            tc.tile_pool(name="zp32", bufs=2) as zpool,
            tc.tile_pool(name="ssp", bufs=2) as sspool,
            tc.tile_pool(name="wp", bufs=2) as wpool,
            tc.tile_pool(name="pp", bufs=2) as ppool_s,
            tc.tile_pool(name="qp", bufs=2) as qpool,
            tc.tile_pool(name="sp", bufs=2) as stpool,
            tc.tile_pool(name="ps", bufs=8, space="PSUM") as ppool,
        ):
            mb16 = cpool.tile([128, 3, 128], f16)
            for mi in range(3):
                nc.sync.dma_start(out=mb16[:, mi, :], in_=mats16[mi, :, :])
            mb32 = cpool.tile([128, 1, 128], f32)
            nc.sync.dma_start(out=mb32[:, 0, :], in_=mats32[0, :, :])
            msb = cpool.tile([128, _ND + 1], f16)
            nc.sync.dma_start(out=msb[:], in_=maskp[:])
            mi_I, mi_nI, mi_M2 = 0, 1, 2

            rep_ctx = (
                tc.For_i(0, repeat, 1) if repeat > 1 else contextlib.nullcontext()
            )
            with rep_ctx:
              for pair in (_CHUNKS[0:2], _CHUNKS[2:4]):
                tiles = {}
                for ch in pair:
                    b, h0, ow0, oh = ch
                    t32 = tpool.tile([128, _ND, _F], f32, tag="t")
                    z32 = zpool.tile([128, _ND, _F], f32, tag="z")
                    ss = sspool.tile([128, _ND, _F], f16, tag="ss")
                    w = wpool.tile([128, _ND, _F], f16, tag="w")
                    zp = ppool_s.tile([128, _ND + 1, _F], f16, tag="zp")
                    zq = qpool.tile([128, _ND, _F + 1], f16, tag="zq")
                    st = stpool.tile([128, _ND, _F], f16, tag="st")
                    nc.sync.dma_start(out=t32[:], in_=img[:, b, :, h0 : h0 + _F])
                    nc.sync.dma_start(out=ss[:], in_=ssd[:, b, :, h0 : h0 + _F])
                    nc.vector.memset(zp[:, 0:1, :], 0.0)
                    nc.vector.memset(zp[:, _ND : _ND + 1, :], 0.0)
                    nc.vector.memset(zq[:, :, 0:1], 0.0)
                    nc.vector.memset(zq[:, :, _F : _F + 1], 0.0)
                    tiles[ch] = (t32, z32, ss, w, zp, zq, st)

                for c in range(_CASC):
                    for ch in pair:
                        t32, z32, ss, w, zp, zq, st = tiles[ch]
                        # ---- z = (1-lamb)*t + ss  (fp32) ----
                        nc.vector.scalar_tensor_tensor(
                            z32[:], t32[:], 1.0 - _LAMB, ss[:], OP.mult, OP.add
                        )
                        # ---- p chain (D axis): w = z[d] - z[d+1] ----
                        nc.vector.tensor_tensor(
                            w[:, 0 : _ND - 1, :],
                            z32[:, 0 : _ND - 1, :],
                            z32[:, 1:_ND, :],
                            OP.subtract,
                        )
                        if c == 0:
                            nc.vector.tensor_scalar(
                                zp[:, 1:_ND, :],
                                w[:, 0 : _ND - 1, :],
                                -s0,
                                s0,
                                OP.max,
                                OP.min,
                            )
                        else:
                            nc.vector.tensor_tensor(
                                zp[:, 1:_ND, :],
                                zp[:, 1:_ND, :],
                                w[:, 0 : _ND - 1, :],
                                OP.add,
                            )
                            nc.vector.tensor_scalar(
                                zp[:, 1:_ND, :], zp[:, 1:_ND, :], -s0, s0,
                                OP.max, OP.min,
                            )
                        # mask out-of-domain dual slabs (gd<0 / gd=255)
                        for e0, e1 in ((1, 4), (35, 36)):
                            nc.vector.tensor_tensor(
                                zp[:, e0:e1, :],
                                zp[:, e0:e1, :],
                                msb[:, e0:e1]
                                .unsqueeze(2)
                                .broadcast_to([128, e1 - e0, _F]),
                                OP.mult,
                            )
                        # ---- q chain (H axis): w = z[h] - z[h+1] ----
                        nc.vector.tensor_tensor(
                            w[:, :, 0 : _F - 1],
                            z32[:, :, 0 : _F - 1],
                            z32[:, :, 1:_F],
                            OP.subtract,
                        )
                        if c == 0:
                            nc.vector.tensor_scalar(
                                zq[:, :, 1:_F],
                                w[:, :, 0 : _F - 1],
                                -s1,
                                s1,
                                OP.max,
                                OP.min,
                            )
                        else:
                            nc.vector.tensor_tensor(
                                zq[:, :, 1:_F],
                                zq[:, :, 1:_F],
                                w[:, :, 0 : _F - 1],
                                OP.add,
                            )
                            nc.vector.tensor_scalar(
                                zq[:, :, 1:_F], zq[:, :, 1:_F], -s1, s1,
                                OP.max, OP.min,
                            )
                        # ---- zn = clip(z) (fp16, into w after q consumed) ----
                        nc.vector.tensor_scalar(
                            w[:], z32[:], -s3, s3, OP.max, OP.min
                        )
                        # ---- s~ chain (W axis on TensorE) ----
                        for g0, g in groups():
                            ps = ppool.tile([128, _MMG, _F], f32, tag="ps")
                            nc.tensor.matmul(
                                ps[:, 0:g, :],
                                mb32[:, 0, :],
                                z32[:, g0 : g0 + g, :],
                                start=True,
                                stop=(c == 0),
                            )
                            if c > 0:
                                nc.tensor.matmul(
                                    ps[:, 0:g, :],
                                    mb16[:, mi_I, :],
                                    st[:, g0 : g0 + g, :],
                                    start=False,
                                    stop=True,
                                )
                            nc.scalar.copy(
                                out=st[:, g0 : g0 + g, :], in_=ps[:, 0:g, :]
                            )
                        nc.vector.tensor_scalar(
                            st[:], st[:], -s2, s2, OP.max, OP.min
                        )
                    for ch in pair:
                        b, h0, ow0, oh = ch
                        t32, z32, ss, w, zp, zq, st = tiles[ch]
                        # ---- t = D_d^T p + D_h^T q + M2 s~ + zn  (PSUM) ----
                        for g0, g in groups():
                            pt = ppool.tile([128, _MMG, _F], f32, tag="ps")
                            nc.tensor.matmul(
                                pt[:, 0:g, :],
                                mb16[:, mi_I, :],
                                w[:, g0 : g0 + g, :],
                                start=True,
                                stop=False,
                            )
                            nc.tensor.matmul(
                                pt[:, 0:g, :],
                                mb16[:, mi_I, :],
                                zp[:, g0 : g0 + g, :],
                                start=False,
                                stop=False,
                            )
                            nc.tensor.matmul(
                                pt[:, 0:g, :],
                                mb16[:, mi_I, :],
                                zq[:, g0 : g0 + g, 0:_F],
                                start=False,
                                stop=False,
                            )
                            nc.tensor.matmul(
                                pt[:, 0:g, :],
                                mb16[:, mi_nI, :],
                                zp[:, g0 + 1 : g0 + g + 1, :],
                                start=False,
                                stop=False,
                            )
                            nc.tensor.matmul(
                                pt[:, 0:g, :],
                                mb16[:, mi_nI, :],
                                zq[:, g0 : g0 + g, 1 : _F + 1],
                                start=False,
                                stop=False,
                            )
                            nc.tensor.matmul(
                                pt[:, 0:g, :],
                                mb16[:, mi_M2, :],
                                st[:, g0 : g0 + g, :],
                                start=False,
                                stop=True,
                            )
                            nc.scalar.copy(
                                out=t32[:, g0 : g0 + g, :], in_=pt[:, 0:g, :]
                            )
                        nc.sync.dma_start(
                            out=outs[c][:, b, :, oh : oh + 128],
                            in_=t32[:, _HALO : _HALO + _DCH, ow0 : ow0 + 128],
                        )
    nc.compile()
    return nc


def _make_runner(nc, n_cores):
    """Build a reusable (cached-jit) runner for the Bass program, modeled
    on concourse.bass2jax.run_bass_via_pjrt."""
    import jax
    from jax.experimental.shard_map import shard_map
    from jax.sharding import Mesh, PartitionSpec

    from concourse import bass2jax, mybir

    bass2jax.install_neuronx_cc_hook()

    partition_name = (
        nc.partition_id_tensor.name if nc.partition_id_tensor else None
    )
    in_names, out_names, out_avals = [], [], []
    for alloc in nc.m.functions[0].allocations:
        if not isinstance(alloc, mybir.MemoryLocationSet):
            continue
        name = alloc.memorylocations[0].name
        if alloc.kind == "ExternalInput":
            if name != partition_name:
                in_names.append(name)
        elif alloc.kind == "ExternalOutput":
            shape = tuple(alloc.tensor_shape)
            dtype = mybir.dt.np(alloc.dtype)
            out_names.append(name)
            out_avals.append(jax.core.ShapedArray(shape, dtype))
    n_params = len(in_names)
    n_outs = len(out_avals)
    all_in_names = tuple(in_names + out_names + ([partition_name] if partition_name else []))
    donate = tuple(range(n_params, n_params + n_outs))

    def _body(*args):
        operands = list(args)
        if partition_name is not None:
            operands.append(bass2jax.partition_id_tensor())
        return tuple(
            bass2jax._bass_exec_p.bind(
                *operands,
                out_avals=tuple(out_avals),
                in_names=all_in_names,
                out_names=tuple(out_names),
                lowering_input_output_aliases=(),
                sim_require_finite=True,
                sim_require_nnan=True,
                nc=nc,
            )
        )

    devices = jax.devices()[:n_cores]
    assert len(devices) == n_cores
    mesh = Mesh(np.asarray(devices), ("core",))
    in_specs = (PartitionSpec("core"),) * (n_params + n_outs)
    out_specs = (PartitionSpec("core"),) * n_outs
    sharded = jax.jit(
        shard_map(
            _body, mesh=mesh, in_specs=in_specs, out_specs=out_specs, check_rep=False
        ),
        donate_argnums=donate,
        keep_unused=True,
    )

    def _concat_inputs(in_maps):
        per_core = [[np.asarray(m[name]) for name in in_names] for m in in_maps]
        return [
            np.concatenate([per_core[c][i] for c in range(n_cores)], axis=0)
            for i in range(n_params)
        ]

    def run(in_maps):
        concat_in = _concat_inputs(in_maps)
        concat_zeros = [
            np.zeros((n_cores * a.shape[0], *a.shape[1:]), a.dtype) for a in out_avals
        ]
        out_arrs = sharded(*concat_in, *concat_zeros)
        return [
            {
                name: np.asarray(out_arrs[i]).reshape(
                    n_cores, *out_avals[i].shape
                )[c]
                for i, name in enumerate(out_names)
            }
            for c in range(n_cores)
        ]

    def time_device(in_maps, reps=20):
        """Device-exec wall time with inputs pre-staged on device and
        outputs left on device (no tunnel transfer in the timed region)."""
        import time as _time

        sharded_nodonate = jax.jit(
            shard_map(
                _body,
                mesh=mesh,
                in_specs=in_specs,
                out_specs=out_specs,
                check_rep=False,
            ),
            keep_unused=True,
        )
        from jax.sharding import NamedSharding

        concat_in = _concat_inputs(in_maps)
        concat_zeros = [
            np.zeros((n_cores * a.shape[0], *a.shape[1:]), a.dtype) for a in out_avals
        ]
        shard = NamedSharding(mesh, PartitionSpec("core"))
        dev_in = [jax.device_put(x, shard) for x in concat_in]
        dev_zero = [jax.device_put(x, shard) for x in concat_zeros]
        out = sharded_nodonate(*dev_in, *dev_zero)  # warm + compile
        jax.block_until_ready(out)
        times = []
        for _ in range(reps):
            t0 = _time.perf_counter()
            out = sharded_nodonate(*dev_in, *dev_zero)
            jax.block_until_ready(out)
            times.append(_time.perf_counter() - t0)
        return times

    run.time_device = time_device
    return run


def _get_runner(sigma):
    key = tuple(float(x) for x in np.asarray(sigma).ravel())
    if key not in _RUNNER_CACHE:
        nc = _build_program(sigma)
        _RUNNER_CACHE[key] = _make_runner(nc, _NCORES)
    return _RUNNER_CACHE[key]


def _build_in_maps(image, sino):
    from concurrent.futures import ThreadPoolExecutor

    m1, m2, ident = _stencil_mats()
    mats16 = np.stack([ident, -ident, m2]).astype(np.float16)
    mats32 = m1[None].astype(np.float32)

    def one_core(k):
        d0 = k * _DCH - _HALO
        img_c = np.zeros((_W, _B, _ND, _H), np.float32)
        ss_c = np.zeros((_W, _B, _ND, _H), np.float16)
        lo, hi = max(0, d0), min(_D, d0 + _ND)
        img_c[:, :, lo - d0 : hi - d0, :] = image[:, lo:hi].transpose(3, 0, 1, 2)
        ss_c[:, :, lo - d0 : hi - d0, :] = (
            _LAMB * sino[:, lo:hi].transpose(3, 0, 1, 2)
        ).astype(np.float16)
        # maskp[j]: dual p at gd = 32k - 4 + j; valid iff 0 <= gd <= 254
        gd = _DCH * k - 4 + np.arange(_ND + 1)
        maskp = np.broadcast_to(
            ((gd >= 0) & (gd <= _D - 2)).astype(np.float16), (128, _ND + 1)
        ).copy()
        return {
            "img": img_c,
            "ssd": ss_c,
            "mats16": mats16,
            "mats32": mats32,
            "maskp": maskp,
        }

    with ThreadPoolExecutor(max_workers=_NCORES) as ex:
        return list(ex.map(one_core, range(_NCORES)))


def _reference_numpy(image, sino, sigma, nt):
    """Slow exact fallback for unexpected inputs (e.g. nt != 0)."""
    def fwd_diff(v, ax):
        d = np.diff(v, axis=ax)
        pad = [(0, 0)] * v.ndim
        pad[ax] = (0, 1)
        return np.pad(d, pad)

    def fwd_diff_t(pp, ax):
        n = pp.shape[ax]
        pad_front = [(0, 0)] * pp.ndim
        pad_front[ax] = (1, 0)
        a = np.pad(pp, pad_front)
        a = np.take(a, range(n), axis=ax)
        pad_back = [(0, 0)] * pp.ndim
        pad_back[ax] = (0, 1)
        b = np.pad(np.take(pp, range(n - 1), axis=ax), pad_back)
        return a - b

    t = image.astype(np.float32)
    out = [t]
    p = np.zeros_like(t)
    q = np.zeros_like(t)
    s = np.zeros_like(t)
    for c in range(_CASC):
        z = t - np.float32(_LAMB) * (t - sino)
        pn = np.clip(p - fwd_diff(z, 1), -sigma[0], sigma[0])
        qn = np.clip(q - fwd_diff(z, 2), -sigma[1], sigma[1])
        sn = np.clip(s - fwd_diff(z, 3), -sigma[2], sigma[2])
        zn = np.clip(z, -sigma[3], sigma[3])
        p = pn + nt[c] * (pn - p)
        q = qn + nt[c] * (qn - q)
        s = sn + nt[c] * (sn - s)
        t = fwd_diff_t(p, 1) + fwd_diff_t(q, 2) + fwd_diff_t(s, 3) + zn
        out.append(t.astype(np.float32))
    return tuple(out)


def kernel(image, sino, sigma, nt):
    image = np.asarray(image, np.float32)
    sino = np.asarray(sino, np.float32)
    sigma = np.asarray(sigma, np.float32)
    nt = np.asarray(nt, np.float32)

    if (
        image.shape != (_B, _D, _H, _W)
        or sino.shape != (_B, _D, _H, _W)
        or np.any(nt != 0.0)
    ):
        return _reference_numpy(image, sino, sigma, nt)

    try:
        return _device_path(image, sino, sigma)
    except Exception:
        try:
            return _device_path(image, sino, sigma)  # retry: transient wedge
        except Exception:
            return _reference_numpy(image, sino, sigma, nt)


def _device_path(image, sino, sigma):
    runner = _get_runner(sigma)
    results = runner(_build_in_maps(image, sino))

    from concurrent.futures import ThreadPoolExecutor

    def gather(c):
        # per-core [W, B, DCH, H] -> concat d -> [B, D, H, W]
        cat = np.concatenate(
            [results[k][f"out{c}"] for k in range(_NCORES)], axis=2
        )
        return np.ascontiguousarray(cat.transpose(1, 2, 3, 0))

    with ThreadPoolExecutor(max_workers=_CASC) as ex:
        full = list(ex.map(gather, range(_CASC)))
    return (image, full[0], full[1], full[2])
